# revision 7
# baseline (speedup 1.0000x reference)
"""Trainium2 Bass kernel for nn_MeanAggregator (GAT-style graph attention).

Self-contained: takes FULL inputs as numpy arrays, shards rows across 8
NeuronCores, runs one SPMD Bass/Tile program, returns the FULL [4096, 128]
output.

Math (head h, a_i = att_s[i,h], b_j = att_n[j,h]):
  exp(leaky_relu(a_i + b_j)) = max(e^x, e^{0.2x}) = e^{0.2x} + relu(e^x - e^{0.2x})
  E[j,i] = A[i,j] * (e^{0.2a_i} e^{0.2b_j} + relu(e^{a_i}e^{b_j} - e^{0.2a_i}e^{0.2b_j}))
  out[i, hd] = relu( (sum_j E[j,i] nf[j,hd]) / (sum_j E[j,i]) )

Distribution strategy (v2):
  - i-rows data-parallel (512 per core), A^T slab per core (bf16, host).
  - neighbor aggregation: the 4096x25 neighbor feature rows are routed to
    cores by (j mod NC is irrelevant -- each core is assigned 1/8 of the
    pair workload, grouped by j-chunk and shipped in pair order as featP).
    On device, 0/1 selection matrices (built from iota + is_equal) turn the
    segment-sum into PE matmuls: G[j,:] = sum_pairs feat[n,:], then
    b = G @ (W @ wn)/25 and ONE AllReduce(add) of [4096, 8] logits.
  - node rows are host-gathered (own_feat), projected on device; nf values
    distributed with one small AllGather.
  - phase B (dense masked attention): diff tiles via K=16 PE matmul,
    relu on ACT/DVE (split), mask-mult on DVE, T2/T1 PSUM accumulation.
"""
import numpy as np
import ml_dtypes

N, NEIGH, F, H, D = 4096, 25, 128, 8, 16
NUM_NODES = 100000
NC = 8
ROWS = N // NC          # 512 rows per core
JC = N // 128           # 32 j-chunks
IC = ROWS // 128        # 4 i-chunks per core
HD = H * D              # 128

_PROGRAMS = {}
_RUNNERS = {}
LAST_EXEC_NS = None
DVE_RELU_MOD = 3        # every 3rd dps tile's relu goes to DVE instead of ACT
NO_CC = False           # replace collectives with local copies (timing aid)


def _build_program(PCJ, repeat=1, no_cc=False):
    """PCJ = pair-chunks per j-chunk (128 pairs each)."""
    import concourse.bass as bass
    import concourse.bacc as bacc
    import concourse.tile as tile
    from concourse import mybir
    from contextlib import ExitStack

    f32 = mybir.dt.float32
    f32r = mybir.dt.float32r
    bf16 = mybir.dt.bfloat16
    i32 = mybir.dt.int32
    AF = mybir.ActivationFunctionType
    ALU = mybir.AluOpType

    PCH = JC * PCJ      # total pair chunks per core

    nc = bacc.Bacc("TRN2", target_bir_lowering=False, debug=False, num_devices=NC)

    featP = nc.declare_dram_parameter("featP", [128, PCH * F], bf16, isOutput=False)
    jt = nc.declare_dram_parameter("jt", [128, PCH], f32, isOutput=False)
    own_feat = nc.declare_dram_parameter("own_feat", [128, IC, F], f32, isOutput=False)
    a_t = nc.declare_dram_parameter("a_t", [N, ROWS], bf16, isOutput=False)
    w_pad = nc.declare_dram_parameter("w_pad", [F, 136], f32, isOutput=False)
    ws_cat = nc.declare_dram_parameter("ws_cat", [F, 16], f32, isOutput=False)
    wn8 = nc.declare_dram_parameter("wn8", [F, H], bf16, isOutput=False)
    ident = nc.declare_dram_parameter("ident", [128, 128], f32, isOutput=False)
    identb = nc.declare_dram_parameter("identb", [128, 128], bf16, isOutput=False)
    sign16 = nc.declare_dram_parameter("sign16", [16, 1], f32, isOutput=False)
    out = nc.declare_dram_parameter("out", [ROWS, HD], f32, isOutput=True)

    with tile.TileContext(nc) as tc, ExitStack() as ctx:
        if repeat > 1:
            ctx.enter_context(tc.For_i(0, repeat, 1))
        # ---- pools
        big = ctx.enter_context(tc.tile_pool(name="big", bufs=1))
        sm = ctx.enter_context(tc.tile_pool(name="sm", bufs=1))
        mrp = ctx.enter_context(tc.tile_pool(name="mrp", bufs=8))
        fsp = ctx.enter_context(tc.tile_pool(name="fsp", bufs=4))
        dps_pool = ctx.enter_context(tc.tile_pool(name="dps", bufs=2, space="PSUM"))
        acc_pool = ctx.enter_context(tc.tile_pool(name="acc", bufs=1, space="PSUM"))
        dram = ctx.enter_context(tc.tile_pool(name="dram", bufs=1, space="DRAM"))

        # ---- constants / small inputs
        wpad_f = sm.tile([F, 136], f32)
        nc.sync.dma_start(out=wpad_f[:], in_=w_pad[:])
        wpad_sb = sm.tile([F, 136], f32r)
        nc.vector.tensor_copy(out=wpad_sb[:], in_=wpad_f[:])
        wsc_f = sm.tile([F, 16], f32)
        nc.sync.dma_start(out=wsc_f[:], in_=ws_cat[:])
        wsc_sb = sm.tile([F, 16], f32r)
        nc.vector.tensor_copy(out=wsc_sb[:], in_=wsc_f[:])
        wn8_sb = sm.tile([F, H], bf16)
        nc.sync.dma_start(out=wn8_sb[:], in_=wn8[:])
        id_sb = sm.tile([128, 128], f32)
        nc.sync.dma_start(out=id_sb[:], in_=ident[:])
        idb_sb = sm.tile([128, 128], bf16)
        nc.sync.dma_start(out=idb_sb[:], in_=identb[:])
        sg_sb = sm.tile([16, 1], f32)
        nc.sync.dma_start(out=sg_sb[:], in_=sign16[:])
        jt_sb = sm.tile([128, PCH], f32)
        nc.sync.dma_start(out=jt_sb[:], in_=jt[:])
        ownbuf = sm.tile([128, IC, F], f32)
        nc.sync.dma_start(out=ownbuf[:], in_=own_feat[:])

        # ---- A^T slab (bf16): a_sb[p, jc, i] = A[own, :].T chunk
        a_sb = big.tile([128, JC, ROWS], bf16)
        nc.sync.dma_start(out=a_sb[:], in_=a_t.rearrange("(c p) i -> p c i", p=128))

        # ---- own-row projections (feeds the nf AllGather)
        ownT = sm.tile([128, IC, 128], f32r)
        for c in range(IC):
            tp = dps_pool.tile([128, 256], f32, tag="dpair", name=f"tpo{c}")
            nc.tensor.transpose(out=tp[:, :128], in_=ownbuf[:, c, :], identity=id_sb[:])
            nc.vector.tensor_copy(out=ownT[:, c, :], in_=tp[:, :128])

        nfl = sm.tile([128, IC, 136], bf16)
        for c in range(IC):
            pp = dps_pool.tile([128, 136], f32, tag="dpair", name=f"pp{c}")
            nc.tensor.matmul(out=pp[:], lhsT=ownT[:, c, :],
                             rhs=wpad_sb[:], start=True, stop=True)
            nc.vector.tensor_copy(out=nfl[:, c, :], in_=pp[:])
        ones_l = bass.AP(
            tensor=nfl.tensor, offset=nfl[:].offset + 16,
            ap=[nfl[:].ap[0], [136, IC], [17, H]],
        )
        nc.vector.memset(ones_l, 1.0)
        nfl_d = dram.tile([ROWS, 136], bf16)
        nc.sync.dma_start(out=nfl_d[:].rearrange("(c p) f -> p c f", p=128), in_=nfl[:])
        nfg_d = dram.tile([N, 136], bf16)
        if no_cc:
            nc.sync.dma_start(out=nfg_d[0:ROWS, :], in_=nfl_d[:])
        else:
            nc.gpsimd.collective_compute(
                "AllGather", ALU.bypass, replica_groups=[list(range(NC))],
                ins=[nfl_d.opt()], outs=[nfg_d.opt()],
            )

        # ---- neighbor aggregation: selection-matmul segment sum.
        # featP is host-transposed: partition p holds its pair rows for all
        # chunks contiguously -> one big DMA.
        featP_sb = big.tile([128, PCH, F], bf16)
        nc.sync.dma_start(out=featP_sb[:], in_=featP.rearrange("p (c f) -> p c f", f=F))
        iota = sm.tile([128, 128], f32)
        nc.gpsimd.iota(out=iota[:], pattern=[[1, 128]], base=0, channel_multiplier=0,
                       allow_small_or_imprecise_dtypes=True)
        b_sb = sm.tile([128, JC, H], f32)
        for jc in range(JC):
            gp = dps_pool.tile([128, 128], f32, tag="dpair", name=f"g{jc}")
            for q in range(PCJ):
                pc = jc * PCJ + q
                sel = fsp.tile([128, 128], bf16, tag="sel", name=f"sel{pc}")
                nc.vector.tensor_scalar(
                    out=sel[:], in0=iota[:], scalar1=jt_sb[:, pc:pc + 1],
                    scalar2=None, op0=ALU.is_equal)
                # out = featP_pc^T @ sel = G^T chunk [f, j]
                nc.tensor.matmul(out=gp[:], lhsT=featP_sb[:, pc, :], rhs=sel[:],
                                 start=(q == 0), stop=(q == PCJ - 1))
            gt = fsp.tile([128, 128], bf16, tag="gt", name=f"gt{jc}")
            nc.scalar.copy(out=gt[:], in_=gp[:])
            bp = dps_pool.tile([128, H], f32, tag="dpair", name=f"bp{jc}")
            nc.tensor.matmul(out=bp[:], lhsT=gt[:], rhs=wn8_sb[:],
                             start=True, stop=True)
            nc.scalar.copy(out=b_sb[:, jc, :], in_=bp[:])
        ar_in = dram.tile([N, H], f32)
        nc.sync.dma_start(out=ar_in[:].rearrange("(c p) h -> p c h", p=128),
                          in_=b_sb[:])
        b_full = dram.tile([N, H], f32)
        if no_cc:
            nc.sync.dma_start(out=b_full[0:128, :], in_=ar_in[0:128, :])
        else:
            nc.gpsimd.collective_compute(
                "AllReduce", ALU.add, replica_groups=[list(range(NC))],
                ins=[ar_in.opt()], outs=[b_full.opt()],
            )

        # ---- att_s: ea16 [16, 512] (rows 0-7 e^a, 8-15 e^{0.2a})
        as_ps = dps_pool.tile([16, 512], f32, tag="dpair", name="as_ps")
        ownT_flat = ownT[:].rearrange("p c f -> p (c f)")
        nc.tensor.matmul(out=as_ps[:], lhsT=wsc_sb[:], rhs=ownT_flat, start=True, stop=True)
        ea16 = sm.tile([16, 512], f32)
        nc.scalar.activation(out=ea16[:], in_=as_ps[:], func=AF.Exp)
        nc.vector.tensor_scalar_mul(ea16[:], ea16[:], sg_sb[:, 0:1])
        ea_bdf = sm.tile([16, H, 512], f32)    # block-diag rhs, zero elsewhere
        nc.vector.memset(ea_bdf[:], 0.0)
        for h in range(H):
            nc.sync.dma_start(out=ea_bdf[2 * h:2 * h + 1, h, :], in_=ea16[h:h + 1, :])
            nc.sync.dma_start(out=ea_bdf[2 * h + 1:2 * h + 2, h, :], in_=ea16[8 + h:9 + h, :])
        ea_bd = sm.tile([16, H, 512], f32r)
        nc.vector.tensor_copy(out=ea_bd[:], in_=ea_bdf[:])

        # e02a natural [128, ic, 8]
        e02an = sm.tile([128, IC, H], f32)
        for ic in range(IC):
            ap8 = dps_pool.tile([128, 8], f32, tag="dpair", name=f"ap8_{ic}")
            nc.tensor.matmul(out=ap8[:], lhsT=ownT[:, ic, :], rhs=wsc_sb[:, 8:16],
                             start=True, stop=True)
            nc.scalar.activation(out=e02an[:, ic, :], in_=ap8[:], func=AF.Exp)

        # ---- nf_pad for all 4096 j (from the AllGather)
        nf_pad = big.tile([128, JC, 136], bf16)
        nc.sync.dma_start(out=nf_pad[:], in_=nfg_d[:].rearrange("(c p) f -> p c f", p=128))

        # ---- post-AllReduce: e^{b}, e^{0.2b} in both layouts
        bn_sb = sm.tile([128, JC, H], f32)
        nc.sync.dma_start(out=bn_sb[:], in_=b_full[:].rearrange("(c p) h -> p c h", p=128))
        e02bn = sm.tile([128, JC, H], f32)
        nc.scalar.activation(out=e02bn[:].rearrange("p c h -> p (c h)"),
                             in_=bn_sb[:].rearrange("p c h -> p (c h)"),
                             func=AF.Exp, scale=0.2)
        ebi = sm.tile([128, JC, 16], f32)
        ebi_even = bass.AP(tensor=ebi.tensor, offset=ebi[:].offset,
                           ap=[ebi[:].ap[0], [16, JC], [2, H]])
        ebi_odd = bass.AP(tensor=ebi.tensor, offset=ebi[:].offset + 1,
                          ap=[ebi[:].ap[0], [16, JC], [2, H]])
        nc.scalar.activation(out=ebi_even, in_=bn_sb[:], func=AF.Exp)
        nc.vector.tensor_copy(out=ebi_odd, in_=e02bn[:])
        # transpose to eb16 [16, 4096] (f32r) interleaved pairs
        eb16 = big.tile([16, N], f32r)
        for g in range(4):
            ebT = dps_pool.tile([16, 1024], f32, tag="dpair", name=f"ebT{g}")
            for q in range(8):
                jc = 8 * g + q
                nc.tensor.transpose(out=ebT[:, 128 * q:128 * q + 128],
                                    in_=ebi[:, jc, :], identity=id_sb[:])
            nc.scalar.copy(out=eb16[:, 1024 * g:1024 * g + 1024], in_=ebT[:])

        # ---- phase B
        t1_ps = [acc_pool.tile([128, 512], f32, tag=f"t1_{i}", name=f"t1_{i}")
                 for i in range(2)]
        t2_ps = [acc_pool.tile([128, 512], f32, tag=f"t2_{i}", name=f"t2_{i}")
                 for i in range(2)]
        for c in range(JC):
            for hp in range(4):
                idx = 4 * c + hp
                dps = dps_pool.tile([128, 1024], f32, tag="dpair", name=f"d{c}_{hp}")
                for t in range(2):
                    h = 2 * hp + t
                    nc.tensor.matmul(
                        out=dps[:, 512 * t:512 * t + 512],
                        lhsT=eb16[:, 128 * c:128 * c + 128],
                        rhs=ea_bd[:, h, :],
                        start=True, stop=True,
                    )
                mr = mrp.tile([128, 1024], bf16, tag="mr", name=f"mr{c}_{hp}")
                nc.scalar.activation(out=mr[:], in_=dps[:], func=AF.Relu)
                mrm = mrp.tile([128, 2, 512], bf16, tag="mrm", name=f"mm{c}_{hp}")
                a_rep = bass.AP(
                    tensor=a_sb.tensor,
                    offset=a_sb[:].offset + c * ROWS,
                    ap=[a_sb[:].ap[0], [0, 2], [1, ROWS]],
                )
                nc.vector.tensor_tensor(
                    out=mrm[:], in0=mr[:].rearrange("p (t i) -> p t i", t=2),
                    in1=a_rep, op=ALU.mult)
                for t in range(2):
                    h = 2 * hp + t
                    nc.tensor.matmul(
                        out=t2_ps[h // 4][32 * (h % 4):32 * (h % 4) + 17, :],
                        lhsT=nf_pad[:, c, 17 * h:17 * h + 17],
                        rhs=mrm[:, t, :],
                        start=(c == 0), stop=(c == JC - 1),
                        tile_position=(0, 32 * (h % 4)),
                        skip_group_check=True,
                    )
            # vp for this chunk, then T1
            vp = mrp.tile([128, H * 17], bf16, tag="vp", name=f"vp{c}")
            e02b_bc = bass.AP(
                tensor=e02bn.tensor,
                offset=e02bn[:].offset + c * H,
                ap=[e02bn[:].ap[0], [1, H], [0, 17]],
            )
            nc.vector.tensor_tensor(
                out=vp[:].rearrange("p (h d) -> p h d", h=H),
                in0=nf_pad[:, c, :].rearrange("p (h d) -> p h d", h=H),
                in1=e02b_bc, op=ALU.mult)
            for ic in range(IC):
                nc.tensor.matmul(
                    out=t1_ps[ic // 2][:, 256 * (ic % 2):256 * (ic % 2) + 136],
                    lhsT=a_sb[:, c, 128 * ic:128 * ic + 128],
                    rhs=vp[:],
                    start=(c == 0 and ic % 2 == 0),
                    stop=(c == JC - 1 and ic % 2 == 1),
                    skip_group_check=True,
                )

        # ---- phase C: epilogue
        t2sb = sm.tile([128, 2, 512], f32)
        for i in range(2):
            for q in range(4):
                nc.vector.tensor_copy(
                    out=t2sb[32 * q:32 * q + 17, i, :],
                    in_=t2_ps[i][32 * q:32 * q + 17, :])
        t17 = sm.tile([17, H, 512], f32)
        for h in range(H):
            nc.sync.dma_start(
                out=t17[:, h, :],
                in_=t2sb[32 * (h % 4):32 * (h % 4) + 17, h // 4, :])
        for ic in range(IC):
            tps = dps_pool.tile([128, 256], f32, tag="dpair", name=f"tp_ep{ic}")
            for h in range(H):
                nc.tensor.transpose(
                    out=tps[:, 17 * h:17 * h + 17],
                    in_=t17[:, h, 128 * ic:128 * ic + 128],
                    identity=id_sb[:17, :17],
                )
            numsb = mrp.tile([128, 136], f32, tag="num", name=f"nm{ic}")
            for h in range(H):
                nc.vector.tensor_scalar_mul(
                    numsb[:, 17 * h:17 * h + 17],
                    t1_ps[ic // 2][:, 256 * (ic % 2) + 17 * h:256 * (ic % 2) + 17 * h + 17],
                    e02an[:, ic, h:h + 1],
                )
            nc.vector.tensor_tensor(out=numsb[:], in0=numsb[:], in1=tps[:, :136], op=ALU.add)
            denr = mrp.tile([128, 8], f32, tag="denr", name=f"dr{ic}")
            den_ap = bass.AP(
                tensor=numsb.tensor, offset=numsb[:].offset + 16,
                ap=[numsb[:].ap[0], [17, H]],
            )
            nc.vector.reciprocal(out=denr[:], in_=den_ap)
            outsb = mrp.tile([128, HD], f32, tag="outsb", name=f"ou{ic}")
            for h in range(H):
                nc.vector.tensor_scalar(
                    out=outsb[:, 16 * h:16 * h + 16],
                    in0=numsb[:, 17 * h:17 * h + 16],
                    scalar1=denr[:, h:h + 1], scalar2=0.0,
                    op0=ALU.mult, op1=ALU.max,
                )
            nc.sync.dma_start(out=out[128 * ic:128 * ic + 128, :], in_=outsb[:])

    nc.compile()
    return nc


def _get_program(PCJ, repeat=1, no_cc=False):
    key = (PCJ, repeat, no_cc)
    if key not in _PROGRAMS:
        _PROGRAMS[key] = _build_program(PCJ, repeat, no_cc)
    return _PROGRAMS[key]


class _Runner:
    """Cached jitted SPMD executor (mirrors bass2jax.run_bass_via_pjrt, but
    builds the jit once and accepts pre-concatenated global arrays)."""

    def __init__(self, nc):
        import jax
        import numpy as _np
        from concourse import mybir
        from concourse.bass2jax import (_bass_exec_p, partition_id_tensor,
                                        install_neuronx_cc_hook)
        from jax.sharding import Mesh, PartitionSpec
        from jax.experimental.shard_map import shard_map

        install_neuronx_cc_hook()
        self.nc = nc
        partition_name = (nc.partition_id_tensor.name
                          if nc.partition_id_tensor else None)
        in_names, out_names, out_avals, zero_outs = [], [], [], []
        for alloc in nc.m.functions[0].allocations:
            if not isinstance(alloc, mybir.MemoryLocationSet):
                continue
            name = alloc.memorylocations[0].name
            if alloc.kind == "ExternalInput":
                if name != partition_name:
                    in_names.append(name)
            elif alloc.kind == "ExternalOutput":
                out_names.append(name)
                shape = tuple(alloc.tensor_shape)
                dtype = mybir.dt.np(alloc.dtype)
                out_avals.append(jax.core.ShapedArray(shape, dtype))
                zero_outs.append(_np.zeros(shape, dtype))
        n_params = len(in_names)
        n_outs = len(out_avals)
        self.in_names = list(in_names)
        self.out_names = out_names
        self.out_avals = out_avals
        self.zero_shapes = [(NC * z.shape[0], *z.shape[1:]) for z in zero_outs]
        self.zero_dtypes = [z.dtype for z in zero_outs]
        all_names = in_names + out_names
        if partition_name is not None:
            all_names = all_names + [partition_name]

        def _body(*args):
            operands = list(args)
            if partition_name is not None:
                operands.append(partition_id_tensor())
            outs = _bass_exec_p.bind(
                *operands, out_avals=tuple(out_avals),
                in_names=tuple(all_names), out_names=tuple(out_names),
                lowering_input_output_aliases=(),
                sim_require_finite=True, sim_require_nnan=True, nc=nc)
            return tuple(outs)

        devices = jax.devices()[:NC]
        mesh = Mesh(_np.asarray(devices), ("core",))
        self.mesh = mesh
        in_specs = (PartitionSpec("core"),) * (n_params + n_outs)
        out_specs = (PartitionSpec("core"),) * n_outs
        self.fn = jax.jit(
            shard_map(_body, mesh=mesh, in_specs=in_specs,
                      out_specs=out_specs, check_rep=False),
            donate_argnums=tuple(range(n_params, n_params + n_outs)),
            keep_unused=True)

    def __call__(self, global_in_map):
        import numpy as _np
        args = [global_in_map[name] for name in self.in_names]
        zeros = [_np.zeros(s, d) for s, d in
                 zip(self.zero_shapes, self.zero_dtypes)]
        outs = self.fn(*args, *zeros)
        return {name: _np.asarray(o) for name, o in zip(self.out_names, outs)}


def _get_runner(PCJ, repeat=1, no_cc=False):
    key = (PCJ, repeat, no_cc)
    if key not in _RUNNERS:
        _RUNNERS[key] = _Runner(_get_program(PCJ, repeat, no_cc))
    return _RUNNERS[key]


def _prep_inputs(A, features, node, neighbor, self_weight, att_self_weight,
                 att_neigh_weight):
    """Build the global (concatenated-over-cores) input arrays."""
    A = np.asarray(A, np.float32)
    features = np.asarray(features, np.float32)
    node = np.asarray(node).astype(np.int64)
    neighbor = np.asarray(neighbor).astype(np.int64)
    W = np.asarray(self_weight, np.float32)
    aw_s = np.asarray(att_self_weight, np.float32).reshape(H, D)
    aw_n = np.asarray(att_neigh_weight, np.float32).reshape(H, D)

    ws_mat = np.zeros((HD, H), np.float32)
    wn_mat = np.zeros((HD, H), np.float32)
    for h in range(H):
        ws_mat[16 * h:16 * h + 16, h] = aw_s[h]
        wn_mat[16 * h:16 * h + 16, h] = aw_n[h]
    Wws = W @ ws_mat
    Wwn = (W @ wn_mat) / NEIGH
    ws_cat = np.concatenate([Wws, 0.2 * Wws], axis=1).astype(np.float32)
    wn8 = Wwn.astype(ml_dtypes.bfloat16)
    w_pad = np.zeros((F, 136), np.float32)
    for h in range(H):
        w_pad[:, 17 * h:17 * h + 16] = W[:, 16 * h:16 * h + 16]
    ident = np.eye(128, dtype=np.float32)
    identb = np.eye(128, dtype=ml_dtypes.bfloat16)
    sign16 = np.concatenate([np.ones((8, 1)), -np.ones((8, 1))]).astype(np.float32)

    # ---- pair routing: each (j, k) pair assigned to core (j*NEIGH+k) % NC?
    # Simpler: round-robin by k-strided split so each core gets ~25/8 per j.
    # Assign pair (j, k) to core k % NC is uneven (k<NEIGH=25). Use a flat
    # round-robin over each j's pairs: core = (j + k) % NC for balance.
    jj_all = np.repeat(np.arange(N), NEIGH)              # [102400]
    kk_all = np.tile(np.arange(NEIGH), N)
    core_of = (jj_all + kk_all) % NC
    featb = features.astype(ml_dtypes.bfloat16)

    # pair-chunks per j-chunk: max pairs any (core, j-chunk) must hold
    jc_all = jj_all // 128
    maxp = 0
    percore = []
    for c in range(NC):
        m = core_of == c
        jjc, kkc = jj_all[m], kk_all[m]
        jcc = jc_all[m]
        cnt = np.bincount(jcc, minlength=JC)
        maxp = max(maxp, int(cnt.max()))
        percore.append((jjc, kkc))
    PCJ = (maxp + 127) // 128
    PCH = JC * PCJ

    # featP is stored transposed: [128 partitions, PCH chunks * F] per core,
    # where partition p of chunk pc holds pair row (pc*128 + p).
    featP_g = np.zeros((NC * 128, PCH * F), ml_dtypes.bfloat16)
    jt_g = np.full((NC * 128, PCH), -1.0, np.float32)
    for c in range(NC):
        jjc, kkc = percore[c]
        nidx = neighbor[jjc, kkc]                        # feature rows
        jcc = jjc // 128
        jloc = jjc % 128
        # position within this j-chunk's pair block
        starts = np.searchsorted(jcc, np.arange(JC))
        rank = np.arange(len(jjc)) - starts[jcc]
        row = jcc * (PCJ * 128) + rank                   # pair row index
        fp = np.zeros((PCH * 128, F), ml_dtypes.bfloat16)
        fp[row] = featb[nidx]
        featP_g[c * 128:(c + 1) * 128] = \
            fp.reshape(PCH, 128, F).transpose(1, 0, 2).reshape(128, PCH * F)
        pc = row // 128
        tloc = row % 128
        jt_g[c * 128 + tloc, pc] = jloc.astype(np.float32)

    # own (node) rows, host-gathered: [NC*128, IC, F]
    own_g = features[node[:, 0]].reshape(NC, IC, 128, F).transpose(0, 2, 1, 3) \
        .reshape(NC * 128, IC, F).astype(np.float32)

    # A^T slabs
    at_g = np.empty((NC * N, ROWS), ml_dtypes.bfloat16)
    for c in range(NC):
        at_g[c * N:(c + 1) * N] = A[c * ROWS:(c + 1) * ROWS, :].T

    small = {
        "w_pad": w_pad, "ws_cat": ws_cat, "wn8": wn8, "ident": ident,
        "identb": identb, "sign16": sign16,
    }
    glob = {
        "featP": featP_g, "jt": jt_g, "own_feat": own_g, "a_t": at_g,
    }
    for k, v in small.items():
        glob[k] = np.concatenate([v] * NC, axis=0)
    return glob, PCJ


def _build_cc_chain(M):
    """Unrolled chain of M (AllGather + AllReduce) rounds for timing."""
    import concourse.bass as bass
    import concourse.bacc as bacc
    import concourse.tile as tile
    from concourse import mybir
    from contextlib import ExitStack

    bf16 = mybir.dt.bfloat16
    ALU = mybir.AluOpType
    nc = bacc.Bacc("TRN2", target_bir_lowering=False, debug=False, num_devices=NC)
    x = nc.declare_dram_parameter("x", [ROWS, 136], bf16, isOutput=False)
    outp = nc.declare_dram_parameter("out", [N, H], bf16, isOutput=True)
    with tile.TileContext(nc) as tc, ExitStack() as ctx:
        dram = ctx.enter_context(tc.tile_pool(name="dram", bufs=1, space="DRAM"))
        x2 = dram.tile([ROWS, 136], bf16)
        nc.sync.dma_start(out=x2[:], in_=x[:])
        y = dram.tile([N, 136], bf16)
        z = dram.tile([N, H], bf16)
        w = dram.tile([N, H], bf16)
        for m in range(M):
            nc.gpsimd.collective_compute(
                "AllGather", ALU.bypass, replica_groups=[list(range(NC))],
                ins=[x2.opt()], outs=[y.opt()])
            nc.sync.dma_start(out=z[:], in_=y[:, 0:H])
            nc.gpsimd.collective_compute(
                "AllReduce", ALU.add, replica_groups=[list(range(NC))],
                ins=[z.opt()], outs=[w.opt()])
            nc.sync.dma_start(out=x2[:, 0:H], in_=w[0:ROWS, :])
        nc.sync.dma_start(out=outp[:], in_=w[:])
    nc.compile()
    return nc


def measure_collectives(m_small=8, m_big=200, samples=12):
    import time, jax
    from jax.sharding import NamedSharding, PartitionSpec
    runners = {}
    dxs = {}
    for M in (m_small, m_big):
        r = _Runner(_build_cc_chain(M))
        x = np.zeros((NC * ROWS, 136), ml_dtypes.bfloat16)
        sh = NamedSharding(r.mesh, PartitionSpec("core"))
        runners[M] = r
        dxs[M] = jax.device_put(x, sh)
    walls = {m_small: [], m_big: []}
    # interleave samples so slow drift in dispatch latency cancels
    for i in range(samples):
        for M in (m_small, m_big):
            r = runners[M]
            zeros = [np.zeros(s, d) for s, d in
                     zip(r.zero_shapes, r.zero_dtypes)]
            t0 = time.time()
            outs = r.fn(dxs[M], *zeros)
            for o in outs:
                o.block_until_ready()
            walls[M].append(time.time() - t0)
    w_small = sorted(walls[m_small])
    w_big = sorted(walls[m_big])
    # median of the lower half is robust to both jitter spikes and min-luck
    lo = max(2, samples // 3)
    est_small = sum(w_small[:lo]) / lo
    est_big = sum(w_big[:lo]) / lo
    return (est_big - est_small) / (m_big - m_small) * 1e9


def kernel(A, features, node, neighbor, self_weight, att_self_weight,
           att_neigh_weight):
    glob, PCJ = _prep_inputs(A, features, node, neighbor, self_weight,
                             att_self_weight, att_neigh_weight)
    runner = _get_runner(PCJ, no_cc=NO_CC)
    res = runner(glob)
    return res["out"].astype(np.float32)


# revision 8
# speedup vs baseline: 1.1477x; 1.1477x over previous
"""Trainium2 Bass kernel for nn_MeanAggregator (GAT-style graph attention).

Self-contained: takes FULL inputs as numpy arrays, shards rows across 8
NeuronCores, runs one SPMD Bass/Tile program, returns the FULL [4096, 128]
output.

Math (head h, a_i = att_s[i,h], b_j = att_n[j,h]):
  exp(leaky_relu(a_i + b_j)) = max(e^x, e^{0.2x}) = e^{0.2x} + relu(e^x - e^{0.2x})
  E[j,i] = A[i,j] * (e^{0.2a_i} e^{0.2b_j} + relu(e^{a_i}e^{b_j} - e^{0.2a_i}e^{0.2b_j}))
  out[i, hd] = relu( (sum_j E[j,i] nf[j,hd]) / (sum_j E[j,i]) )

Distribution strategy (v2):
  - i-rows data-parallel (512 per core), A^T slab per core (bf16, host).
  - neighbor aggregation: the 4096x25 neighbor feature rows are routed to
    cores by (j mod NC is irrelevant -- each core is assigned 1/8 of the
    pair workload, grouped by j-chunk and shipped in pair order as featP).
    On device, 0/1 selection matrices (built from iota + is_equal) turn the
    segment-sum into PE matmuls: G[j,:] = sum_pairs feat[n,:], then
    b = G @ (W @ wn)/25 and ONE AllReduce(add) of [4096, 8] logits.
  - node rows are host-gathered (own_feat), projected on device; nf values
    distributed with one small AllGather.
  - phase B (dense masked attention): diff tiles via K=16 PE matmul,
    relu on ACT/DVE (split), mask-mult on DVE, T2/T1 PSUM accumulation.
"""
import numpy as np
import ml_dtypes

N, NEIGH, F, H, D = 4096, 25, 128, 8, 16
NUM_NODES = 100000
NC = 8
ROWS = N // NC          # 512 rows per core
JC = N // 128           # 32 j-chunks
IC = ROWS // 128        # 4 i-chunks per core
HD = H * D              # 128

_PROGRAMS = {}
_RUNNERS = {}
LAST_EXEC_NS = None
DVE_RELU_MOD = 3        # every 3rd dps tile's relu goes to DVE instead of ACT
NO_CC = False           # replace collectives with local copies (timing aid)


def _build_program(PCJ, repeat=1, no_cc=False):
    """PCJ = pair-chunks per j-chunk (128 pairs each)."""
    import concourse.bass as bass
    import concourse.bacc as bacc
    import concourse.tile as tile
    from concourse import mybir
    from contextlib import ExitStack

    f32 = mybir.dt.float32
    f32r = mybir.dt.float32r
    bf16 = mybir.dt.bfloat16
    i32 = mybir.dt.int32
    AF = mybir.ActivationFunctionType
    ALU = mybir.AluOpType

    PCH = JC * PCJ      # total pair chunks per core

    nc = bacc.Bacc("TRN2", target_bir_lowering=False, debug=False, num_devices=NC)

    featP = nc.declare_dram_parameter("featP", [128, PCH * F], bf16, isOutput=False)
    jt = nc.declare_dram_parameter("jt", [128, PCH], f32, isOutput=False)
    own_feat = nc.declare_dram_parameter("own_feat", [128, IC, F], f32, isOutput=False)
    a_t = nc.declare_dram_parameter("a_t", [N, ROWS], bf16, isOutput=False)
    w_pad = nc.declare_dram_parameter("w_pad", [F, 136], f32, isOutput=False)
    ws_cat = nc.declare_dram_parameter("ws_cat", [F, 16], f32, isOutput=False)
    wn8 = nc.declare_dram_parameter("wn8", [F, H], bf16, isOutput=False)
    ident = nc.declare_dram_parameter("ident", [128, 128], f32, isOutput=False)
    identb = nc.declare_dram_parameter("identb", [128, 128], bf16, isOutput=False)
    sign16 = nc.declare_dram_parameter("sign16", [16, 1], f32, isOutput=False)
    out = nc.declare_dram_parameter("out", [ROWS, HD], f32, isOutput=True)

    with tile.TileContext(nc) as tc, ExitStack() as ctx:
        if repeat > 1:
            ctx.enter_context(tc.For_i(0, repeat, 1))
        # ---- pools
        big = ctx.enter_context(tc.tile_pool(name="big", bufs=1))
        sm = ctx.enter_context(tc.tile_pool(name="sm", bufs=1))
        mrp = ctx.enter_context(tc.tile_pool(name="mrp", bufs=8))
        fsp = ctx.enter_context(tc.tile_pool(name="fsp", bufs=4))
        dps_pool = ctx.enter_context(tc.tile_pool(name="dps", bufs=2, space="PSUM"))
        acc_pool = ctx.enter_context(tc.tile_pool(name="acc", bufs=1, space="PSUM"))
        dram = ctx.enter_context(tc.tile_pool(name="dram", bufs=1, space="DRAM"))

        # ---- constants / small inputs
        wpad_f = sm.tile([F, 136], f32)
        nc.sync.dma_start(out=wpad_f[:], in_=w_pad[:])
        wpad_sb = sm.tile([F, 136], f32r)
        nc.vector.tensor_copy(out=wpad_sb[:], in_=wpad_f[:])
        wsc_f = sm.tile([F, 16], f32)
        nc.sync.dma_start(out=wsc_f[:], in_=ws_cat[:])
        wsc_sb = sm.tile([F, 16], f32r)
        nc.vector.tensor_copy(out=wsc_sb[:], in_=wsc_f[:])
        wn8_sb = sm.tile([F, H], bf16)
        nc.sync.dma_start(out=wn8_sb[:], in_=wn8[:])
        id_sb = sm.tile([128, 128], f32)
        nc.sync.dma_start(out=id_sb[:], in_=ident[:])
        idb_sb = sm.tile([128, 128], bf16)
        nc.sync.dma_start(out=idb_sb[:], in_=identb[:])
        sg_sb = sm.tile([16, 1], f32)
        nc.sync.dma_start(out=sg_sb[:], in_=sign16[:])
        jt_sb = sm.tile([128, PCH], f32)
        nc.sync.dma_start(out=jt_sb[:], in_=jt[:])
        ownbuf = sm.tile([128, IC, F], f32)
        nc.sync.dma_start(out=ownbuf[:], in_=own_feat[:])

        # ---- A^T slab (bf16): a_sb[p, jc, i] = A[own, :].T chunk
        a_sb = big.tile([128, JC, ROWS], bf16)
        nc.sync.dma_start(out=a_sb[:], in_=a_t.rearrange("(c p) i -> p c i", p=128))

        # ---- own-row projections (feeds the nf AllGather)
        ownT = sm.tile([128, IC, 128], f32r)
        for c in range(IC):
            tp = dps_pool.tile([128, 256], f32, tag="dpair", name=f"tpo{c}")
            nc.tensor.transpose(out=tp[:, :128], in_=ownbuf[:, c, :], identity=id_sb[:])
            nc.vector.tensor_copy(out=ownT[:, c, :], in_=tp[:, :128])

        nfl = sm.tile([128, IC, 136], bf16)
        for c in range(IC):
            pp = dps_pool.tile([128, 136], f32, tag="dpair", name=f"pp{c}")
            nc.tensor.matmul(out=pp[:], lhsT=ownT[:, c, :],
                             rhs=wpad_sb[:], start=True, stop=True)
            nc.vector.tensor_copy(out=nfl[:, c, :], in_=pp[:])
        ones_l = bass.AP(
            tensor=nfl.tensor, offset=nfl[:].offset + 16,
            ap=[nfl[:].ap[0], [136, IC], [17, H]],
        )
        nc.vector.memset(ones_l, 1.0)
        nfl_d = dram.tile([ROWS, 136], bf16)
        nc.sync.dma_start(out=nfl_d[:].rearrange("(c p) f -> p c f", p=128), in_=nfl[:])
        nfg_d = dram.tile([N, 136], bf16)
        if no_cc:
            nc.sync.dma_start(out=nfg_d[0:ROWS, :], in_=nfl_d[:])
        else:
            nc.gpsimd.collective_compute(
                "AllGather", ALU.bypass, replica_groups=[list(range(NC))],
                ins=[nfl_d.opt()], outs=[nfg_d.opt()],
            )

        # ---- neighbor aggregation: selection-matmul segment sum.
        # featP is host-transposed: partition p holds its pair rows for all
        # chunks contiguously -> one big DMA.
        featP_sb = big.tile([128, PCH, F], bf16)
        nc.sync.dma_start(out=featP_sb[:], in_=featP.rearrange("p (c f) -> p c f", f=F))
        iota = sm.tile([128, 128], f32)
        nc.gpsimd.iota(out=iota[:], pattern=[[1, 128]], base=0, channel_multiplier=0,
                       allow_small_or_imprecise_dtypes=True)
        b_sb = sm.tile([128, JC, H], f32)
        for jc in range(JC):
            gp = dps_pool.tile([128, 128], f32, tag="dpair", name=f"g{jc}")
            for q in range(PCJ):
                pc = jc * PCJ + q
                sel = fsp.tile([128, 128], bf16, tag="sel", name=f"sel{pc}")
                nc.vector.tensor_scalar(
                    out=sel[:], in0=iota[:], scalar1=jt_sb[:, pc:pc + 1],
                    scalar2=None, op0=ALU.is_equal)
                # out = featP_pc^T @ sel = G^T chunk [f, j]
                nc.tensor.matmul(out=gp[:], lhsT=featP_sb[:, pc, :], rhs=sel[:],
                                 start=(q == 0), stop=(q == PCJ - 1))
            gt = fsp.tile([128, 128], bf16, tag="gt", name=f"gt{jc}")
            nc.scalar.copy(out=gt[:], in_=gp[:])
            bp = dps_pool.tile([128, H], f32, tag="dpair", name=f"bp{jc}")
            nc.tensor.matmul(out=bp[:], lhsT=gt[:], rhs=wn8_sb[:],
                             start=True, stop=True)
            nc.scalar.copy(out=b_sb[:, jc, :], in_=bp[:])
        ar_in = dram.tile([N, H], f32)
        nc.sync.dma_start(out=ar_in[:].rearrange("(c p) h -> p c h", p=128),
                          in_=b_sb[:])
        b_full = dram.tile([N, H], f32)
        if no_cc:
            nc.sync.dma_start(out=b_full[0:128, :], in_=ar_in[0:128, :])
        else:
            nc.gpsimd.collective_compute(
                "AllReduce", ALU.add, replica_groups=[list(range(NC))],
                ins=[ar_in.opt()], outs=[b_full.opt()],
            )

        # ---- att_s: ea16 [16, 512] (rows 0-7 e^a, 8-15 e^{0.2a})
        as_ps = dps_pool.tile([16, 512], f32, tag="dpair", name="as_ps")
        ownT_flat = ownT[:].rearrange("p c f -> p (c f)")
        nc.tensor.matmul(out=as_ps[:], lhsT=wsc_sb[:], rhs=ownT_flat, start=True, stop=True)
        ea16 = sm.tile([16, 512], f32)
        nc.scalar.activation(out=ea16[:], in_=as_ps[:], func=AF.Exp)
        nc.vector.tensor_scalar_mul(ea16[:], ea16[:], sg_sb[:, 0:1])
        ea_bdf = sm.tile([16, H, 512], f32)    # block-diag rhs, zero elsewhere
        nc.vector.memset(ea_bdf[:], 0.0)
        for h in range(H):
            nc.sync.dma_start(out=ea_bdf[2 * h:2 * h + 1, h, :], in_=ea16[h:h + 1, :])
            nc.sync.dma_start(out=ea_bdf[2 * h + 1:2 * h + 2, h, :], in_=ea16[8 + h:9 + h, :])
        ea_bd = sm.tile([16, H, 512], f32r)
        nc.vector.tensor_copy(out=ea_bd[:], in_=ea_bdf[:])

        # e02a natural [128, ic, 8]
        e02an = sm.tile([128, IC, H], f32)
        for ic in range(IC):
            ap8 = dps_pool.tile([128, 8], f32, tag="dpair", name=f"ap8_{ic}")
            nc.tensor.matmul(out=ap8[:], lhsT=ownT[:, ic, :], rhs=wsc_sb[:, 8:16],
                             start=True, stop=True)
            nc.scalar.activation(out=e02an[:, ic, :], in_=ap8[:], func=AF.Exp)

        # ---- nf_pad for all 4096 j (from the AllGather)
        nf_pad = big.tile([128, JC, 136], bf16)
        nc.sync.dma_start(out=nf_pad[:], in_=nfg_d[:].rearrange("(c p) f -> p c f", p=128))

        # ---- post-AllReduce: e^{b}, e^{0.2b} in both layouts
        bn_sb = sm.tile([128, JC, H], f32)
        nc.sync.dma_start(out=bn_sb[:], in_=b_full[:].rearrange("(c p) h -> p c h", p=128))
        e02bn = sm.tile([128, JC, H], f32)
        nc.scalar.activation(out=e02bn[:].rearrange("p c h -> p (c h)"),
                             in_=bn_sb[:].rearrange("p c h -> p (c h)"),
                             func=AF.Exp, scale=0.2)
        ebi = sm.tile([128, JC, 16], f32)
        ebi_even = bass.AP(tensor=ebi.tensor, offset=ebi[:].offset,
                           ap=[ebi[:].ap[0], [16, JC], [2, H]])
        ebi_odd = bass.AP(tensor=ebi.tensor, offset=ebi[:].offset + 1,
                          ap=[ebi[:].ap[0], [16, JC], [2, H]])
        nc.scalar.activation(out=ebi_even, in_=bn_sb[:], func=AF.Exp)
        nc.vector.tensor_copy(out=ebi_odd, in_=e02bn[:])
        # transpose to eb16 [16, 4096] (f32r) interleaved pairs
        eb16 = big.tile([16, N], f32r)
        for g in range(4):
            ebT = dps_pool.tile([16, 1024], f32, tag="dpair", name=f"ebT{g}")
            for q in range(8):
                jc = 8 * g + q
                nc.tensor.transpose(out=ebT[:, 128 * q:128 * q + 128],
                                    in_=ebi[:, jc, :], identity=id_sb[:])
            nc.scalar.copy(out=eb16[:, 1024 * g:1024 * g + 1024], in_=ebT[:])

        # ---- phase B
        t1_ps = [acc_pool.tile([128, 512], f32, tag=f"t1_{i}", name=f"t1_{i}")
                 for i in range(2)]
        t2_ps = [acc_pool.tile([128, 512], f32, tag=f"t2_{i}", name=f"t2_{i}")
                 for i in range(2)]
        for c in range(JC):
            for hp in range(4):
                idx = 4 * c + hp
                dps = dps_pool.tile([128, 1024], f32, tag="dpair", name=f"d{c}_{hp}")
                for t in range(2):
                    h = 2 * hp + t
                    nc.tensor.matmul(
                        out=dps[:, 512 * t:512 * t + 512],
                        lhsT=eb16[:, 128 * c:128 * c + 128],
                        rhs=ea_bd[:, h, :],
                        start=True, stop=True,
                    )
                mr = mrp.tile([128, 1024], bf16, tag="mr", name=f"mr{c}_{hp}")
                nc.scalar.activation(out=mr[:], in_=dps[:], func=AF.Relu)
                mrm = mrp.tile([128, 2, 512], bf16, tag="mrm", name=f"mm{c}_{hp}")
                a_rep = bass.AP(
                    tensor=a_sb.tensor,
                    offset=a_sb[:].offset + c * ROWS,
                    ap=[a_sb[:].ap[0], [0, 2], [1, ROWS]],
                )
                nc.vector.tensor_tensor(
                    out=mrm[:], in0=mr[:].rearrange("p (t i) -> p t i", t=2),
                    in1=a_rep, op=ALU.mult)
                for t in range(2):
                    h = 2 * hp + t
                    nc.tensor.matmul(
                        out=t2_ps[h // 4][32 * (h % 4):32 * (h % 4) + 17, :],
                        lhsT=nf_pad[:, c, 17 * h:17 * h + 17],
                        rhs=mrm[:, t, :],
                        start=(c == 0), stop=(c == JC - 1),
                        tile_position=(0, 32 * (h % 4)),
                        skip_group_check=True,
                    )
            # vp for this chunk, then T1
            vp = mrp.tile([128, H * 17], bf16, tag="vp", name=f"vp{c}")
            e02b_bc = bass.AP(
                tensor=e02bn.tensor,
                offset=e02bn[:].offset + c * H,
                ap=[e02bn[:].ap[0], [1, H], [0, 17]],
            )
            nc.vector.tensor_tensor(
                out=vp[:].rearrange("p (h d) -> p h d", h=H),
                in0=nf_pad[:, c, :].rearrange("p (h d) -> p h d", h=H),
                in1=e02b_bc, op=ALU.mult)
            for ic in range(IC):
                nc.tensor.matmul(
                    out=t1_ps[ic // 2][:, 256 * (ic % 2):256 * (ic % 2) + 136],
                    lhsT=a_sb[:, c, 128 * ic:128 * ic + 128],
                    rhs=vp[:],
                    start=(c == 0 and ic % 2 == 0),
                    stop=(c == JC - 1 and ic % 2 == 1),
                    skip_group_check=True,
                )

        # ---- phase C: epilogue
        t2sb = sm.tile([128, 2, 512], f32)
        for i in range(2):
            for q in range(4):
                nc.vector.tensor_copy(
                    out=t2sb[32 * q:32 * q + 17, i, :],
                    in_=t2_ps[i][32 * q:32 * q + 17, :])
        t17 = sm.tile([17, H, 512], f32)
        for h in range(H):
            nc.sync.dma_start(
                out=t17[:, h, :],
                in_=t2sb[32 * (h % 4):32 * (h % 4) + 17, h // 4, :])
        for ic in range(IC):
            tps = dps_pool.tile([128, 256], f32, tag="dpair", name=f"tp_ep{ic}")
            for h in range(H):
                nc.tensor.transpose(
                    out=tps[:, 17 * h:17 * h + 17],
                    in_=t17[:, h, 128 * ic:128 * ic + 128],
                    identity=id_sb[:17, :17],
                )
            numsb = mrp.tile([128, 136], f32, tag="num", name=f"nm{ic}")
            for h in range(H):
                nc.vector.tensor_scalar_mul(
                    numsb[:, 17 * h:17 * h + 17],
                    t1_ps[ic // 2][:, 256 * (ic % 2) + 17 * h:256 * (ic % 2) + 17 * h + 17],
                    e02an[:, ic, h:h + 1],
                )
            nc.vector.tensor_tensor(out=numsb[:], in0=numsb[:], in1=tps[:, :136], op=ALU.add)
            denr = mrp.tile([128, 8], f32, tag="denr", name=f"dr{ic}")
            den_ap = bass.AP(
                tensor=numsb.tensor, offset=numsb[:].offset + 16,
                ap=[numsb[:].ap[0], [17, H]],
            )
            nc.vector.reciprocal(out=denr[:], in_=den_ap)
            outsb = mrp.tile([128, HD], f32, tag="outsb", name=f"ou{ic}")
            for h in range(H):
                nc.vector.tensor_scalar(
                    out=outsb[:, 16 * h:16 * h + 16],
                    in0=numsb[:, 17 * h:17 * h + 16],
                    scalar1=denr[:, h:h + 1], scalar2=0.0,
                    op0=ALU.mult, op1=ALU.max,
                )
            nc.sync.dma_start(out=out[128 * ic:128 * ic + 128, :], in_=outsb[:])

    nc.compile()
    return nc


def _get_program(PCJ, repeat=1, no_cc=False):
    key = (PCJ, repeat, no_cc)
    if key not in _PROGRAMS:
        _PROGRAMS[key] = _build_program(PCJ, repeat, no_cc)
    return _PROGRAMS[key]


class _Runner:
    """Cached jitted SPMD executor (mirrors bass2jax.run_bass_via_pjrt, but
    builds the jit once and accepts pre-concatenated global arrays)."""

    def __init__(self, nc):
        import jax
        import numpy as _np
        from concourse import mybir
        from concourse.bass2jax import (_bass_exec_p, partition_id_tensor,
                                        install_neuronx_cc_hook)
        from jax.sharding import Mesh, PartitionSpec
        from jax.experimental.shard_map import shard_map

        install_neuronx_cc_hook()
        self.nc = nc
        partition_name = (nc.partition_id_tensor.name
                          if nc.partition_id_tensor else None)
        in_names, out_names, out_avals, zero_outs = [], [], [], []
        for alloc in nc.m.functions[0].allocations:
            if not isinstance(alloc, mybir.MemoryLocationSet):
                continue
            name = alloc.memorylocations[0].name
            if alloc.kind == "ExternalInput":
                if name != partition_name:
                    in_names.append(name)
            elif alloc.kind == "ExternalOutput":
                out_names.append(name)
                shape = tuple(alloc.tensor_shape)
                dtype = mybir.dt.np(alloc.dtype)
                out_avals.append(jax.core.ShapedArray(shape, dtype))
                zero_outs.append(_np.zeros(shape, dtype))
        n_params = len(in_names)
        n_outs = len(out_avals)
        self.in_names = list(in_names)
        self.out_names = out_names
        self.out_avals = out_avals
        self.zero_shapes = [(NC * z.shape[0], *z.shape[1:]) for z in zero_outs]
        self.zero_dtypes = [z.dtype for z in zero_outs]
        all_names = in_names + out_names
        if partition_name is not None:
            all_names = all_names + [partition_name]

        def _body(*args):
            operands = list(args)
            if partition_name is not None:
                operands.append(partition_id_tensor())
            outs = _bass_exec_p.bind(
                *operands, out_avals=tuple(out_avals),
                in_names=tuple(all_names), out_names=tuple(out_names),
                lowering_input_output_aliases=(),
                sim_require_finite=True, sim_require_nnan=True, nc=nc)
            return tuple(outs)

        devices = jax.devices()[:NC]
        mesh = Mesh(_np.asarray(devices), ("core",))
        self.mesh = mesh
        in_specs = (PartitionSpec("core"),) * (n_params + n_outs)
        out_specs = (PartitionSpec("core"),) * n_outs
        self.fn = jax.jit(
            shard_map(_body, mesh=mesh, in_specs=in_specs,
                      out_specs=out_specs, check_rep=False),
            donate_argnums=tuple(range(n_params, n_params + n_outs)),
            keep_unused=True)

    def __call__(self, global_in_map):
        import numpy as _np
        args = [global_in_map[name] for name in self.in_names]
        zeros = [_np.zeros(s, d) for s, d in
                 zip(self.zero_shapes, self.zero_dtypes)]
        outs = self.fn(*args, *zeros)
        return {name: _np.asarray(o) for name, o in zip(self.out_names, outs)}


def _get_runner(PCJ, repeat=1, no_cc=False):
    key = (PCJ, repeat, no_cc)
    if key not in _RUNNERS:
        _RUNNERS[key] = _Runner(_get_program(PCJ, repeat, no_cc))
    return _RUNNERS[key]


def _prep_inputs(A, features, node, neighbor, self_weight, att_self_weight,
                 att_neigh_weight):
    """Build the global (concatenated-over-cores) input arrays."""
    A = np.asarray(A, np.float32)
    features = np.asarray(features, np.float32)
    node = np.asarray(node).astype(np.int64)
    neighbor = np.asarray(neighbor).astype(np.int64)
    W = np.asarray(self_weight, np.float32)
    aw_s = np.asarray(att_self_weight, np.float32).reshape(H, D)
    aw_n = np.asarray(att_neigh_weight, np.float32).reshape(H, D)

    ws_mat = np.zeros((HD, H), np.float32)
    wn_mat = np.zeros((HD, H), np.float32)
    for h in range(H):
        ws_mat[16 * h:16 * h + 16, h] = aw_s[h]
        wn_mat[16 * h:16 * h + 16, h] = aw_n[h]
    Wws = W @ ws_mat
    Wwn = (W @ wn_mat) / NEIGH
    ws_cat = np.concatenate([Wws, 0.2 * Wws], axis=1).astype(np.float32)
    wn8 = Wwn.astype(ml_dtypes.bfloat16)
    w_pad = np.zeros((F, 136), np.float32)
    for h in range(H):
        w_pad[:, 17 * h:17 * h + 16] = W[:, 16 * h:16 * h + 16]
    ident = np.eye(128, dtype=np.float32)
    identb = np.eye(128, dtype=ml_dtypes.bfloat16)
    sign16 = np.concatenate([np.ones((8, 1)), -np.ones((8, 1))]).astype(np.float32)

    # ---- pair routing: each (j, k) pair assigned to core (j*NEIGH+k) % NC?
    # Simpler: round-robin by k-strided split so each core gets ~25/8 per j.
    # Assign pair (j, k) to core k % NC is uneven (k<NEIGH=25). Use a flat
    # round-robin over each j's pairs: core = (j + k) % NC for balance.
    jj_all = np.repeat(np.arange(N), NEIGH)              # [102400]
    kk_all = np.tile(np.arange(NEIGH), N)
    core_of = (jj_all + kk_all) % NC
    featb = features.astype(ml_dtypes.bfloat16)

    # pair-chunks per j-chunk: max pairs any (core, j-chunk) must hold
    jc_all = jj_all // 128
    maxp = 0
    percore = []
    for c in range(NC):
        m = core_of == c
        jjc, kkc = jj_all[m], kk_all[m]
        jcc = jc_all[m]
        cnt = np.bincount(jcc, minlength=JC)
        maxp = max(maxp, int(cnt.max()))
        percore.append((jjc, kkc))
    PCJ = (maxp + 127) // 128
    PCH = JC * PCJ

    # featP is stored transposed: [128 partitions, PCH chunks * F] per core,
    # where partition p of chunk pc holds pair row (pc*128 + p).
    featP_g = np.zeros((NC * 128, PCH * F), ml_dtypes.bfloat16)
    jt_g = np.full((NC * 128, PCH), -1.0, np.float32)
    for c in range(NC):
        jjc, kkc = percore[c]
        nidx = neighbor[jjc, kkc]                        # feature rows
        jcc = jjc // 128
        jloc = jjc % 128
        # position within this j-chunk's pair block
        starts = np.searchsorted(jcc, np.arange(JC))
        rank = np.arange(len(jjc)) - starts[jcc]
        row = jcc * (PCJ * 128) + rank                   # pair row index
        fp = np.zeros((PCH * 128, F), ml_dtypes.bfloat16)
        fp[row] = featb[nidx]
        featP_g[c * 128:(c + 1) * 128] = \
            fp.reshape(PCH, 128, F).transpose(1, 0, 2).reshape(128, PCH * F)
        pc = row // 128
        tloc = row % 128
        jt_g[c * 128 + tloc, pc] = jloc.astype(np.float32)

    # own (node) rows, host-gathered: [NC*128, IC, F]
    own_g = features[node[:, 0]].reshape(NC, IC, 128, F).transpose(0, 2, 1, 3) \
        .reshape(NC * 128, IC, F).astype(np.float32)

    # A^T slabs
    at_g = np.empty((NC * N, ROWS), ml_dtypes.bfloat16)
    for c in range(NC):
        at_g[c * N:(c + 1) * N] = A[c * ROWS:(c + 1) * ROWS, :].T

    small = {
        "w_pad": w_pad, "ws_cat": ws_cat, "wn8": wn8, "ident": ident,
        "identb": identb, "sign16": sign16,
    }
    glob = {
        "featP": featP_g, "jt": jt_g, "own_feat": own_g, "a_t": at_g,
    }
    for k, v in small.items():
        glob[k] = np.concatenate([v] * NC, axis=0)
    return glob, PCJ


def _build_cc_chain(M):
    """Unrolled chain of M (AllGather + AllReduce) rounds for timing."""
    import concourse.bass as bass
    import concourse.bacc as bacc
    import concourse.tile as tile
    from concourse import mybir
    from contextlib import ExitStack

    bf16 = mybir.dt.bfloat16
    ALU = mybir.AluOpType
    nc = bacc.Bacc("TRN2", target_bir_lowering=False, debug=False, num_devices=NC)
    x = nc.declare_dram_parameter("x", [ROWS, 136], bf16, isOutput=False)
    outp = nc.declare_dram_parameter("out", [N, H], bf16, isOutput=True)
    with tile.TileContext(nc) as tc, ExitStack() as ctx:
        dram = ctx.enter_context(tc.tile_pool(name="dram", bufs=1, space="DRAM"))
        x2 = dram.tile([ROWS, 136], bf16)
        nc.sync.dma_start(out=x2[:], in_=x[:])
        y = dram.tile([N, 136], bf16)
        z = dram.tile([N, H], bf16)
        nc.sync.dma_start(out=z[0:ROWS, :], in_=x2[:, 0:H])
        w = dram.tile([N, H], bf16)
        # back-to-back rounds, ordered by queue issue + WAW on y/w — matches
        # how the kernel's own two collectives are serialized, without the
        # artificial data-chaining DMAs of the previous version
        for m in range(M):
            nc.gpsimd.collective_compute(
                "AllGather", ALU.bypass, replica_groups=[list(range(NC))],
                ins=[x2.opt()], outs=[y.opt()])
            nc.gpsimd.collective_compute(
                "AllReduce", ALU.add, replica_groups=[list(range(NC))],
                ins=[z.opt()], outs=[w.opt()])
        nc.sync.dma_start(out=outp[:], in_=w[:])
    nc.compile()
    return nc


def measure_collectives(m_small=8, m_big=200, samples=12):
    import time, jax
    from jax.sharding import NamedSharding, PartitionSpec
    runners = {}
    dxs = {}
    for M in (m_small, m_big):
        r = _Runner(_build_cc_chain(M))
        x = np.zeros((NC * ROWS, 136), ml_dtypes.bfloat16)
        sh = NamedSharding(r.mesh, PartitionSpec("core"))
        runners[M] = r
        dxs[M] = jax.device_put(x, sh)
    walls = {m_small: [], m_big: []}
    # interleave samples so slow drift in dispatch latency cancels
    for i in range(samples):
        for M in (m_small, m_big):
            r = runners[M]
            zeros = [np.zeros(s, d) for s, d in
                     zip(r.zero_shapes, r.zero_dtypes)]
            t0 = time.time()
            outs = r.fn(dxs[M], *zeros)
            for o in outs:
                o.block_until_ready()
            walls[M].append(time.time() - t0)
    w_small = sorted(walls[m_small])
    w_big = sorted(walls[m_big])
    # median of the lower half is robust to both jitter spikes and min-luck
    lo = max(2, samples // 3)
    est_small = sum(w_small[:lo]) / lo
    est_big = sum(w_big[:lo]) / lo
    return (est_big - est_small) / (m_big - m_small) * 1e9


def kernel(A, features, node, neighbor, self_weight, att_self_weight,
           att_neigh_weight):
    glob, PCJ = _prep_inputs(A, features, node, neighbor, self_weight,
                             att_self_weight, att_neigh_weight)
    runner = _get_runner(PCJ, no_cc=NO_CC)
    res = runner(glob)
    return res["out"].astype(np.float32)
